# revision 29
# baseline (speedup 1.0000x reference)
"""LocalWindowAttention Trainium2 kernel (Bass/Tile), 8-core SPMD — v2.

Problem: x[B=4, S=4096, E=512] -> out[B, S, E]
  qkv = x @ W_qkv + b_qkv ; q,k,v = split(qkv)
  scores = (q @ k.T) / sqrt(E), banded mask |i-j| <= 64, softmax
  out = (attn @ v) @ W_out + b_out

Sharding: 8 cores = (batch b in 0..3) x (seq half h in 0..1). Each core owns
2048 query rows and loads a 64-row halo of x on each side (zero-padded at
sequence boundaries), computing q/k/v locally — no collectives.

v2 design (vs the fp32r v1):
  - All matmul operands are bf16 (PSUM accumulation stays fp32). bf16
    streams 1 col/cycle at ANY moving size (fp32r needs >= 256) and gets
    fast weight loads, halving LDWEIGHTS cost.
  - Scores are computed directly TRANSPOSED, per 128-key chunk j:
    scoresT[k in chunk j, q in its 256-query span] = kT_chunk.T @ qT.
    This kills all 32 PE transposes + their PSUM round trips of v1.
  - Softmax without max-subtraction (scores are O(1)); additive band mask
    on DVE in-place in PSUM, then ACT exp -> bf16 exp tile.
  - Row sums via PE: ones-column matmuls with the exp tiles as stationary
    give rowsum[q, 1] directly in natural orientation; DVE reciprocal.
  - Attention is left UNNORMALIZED through attendedT and the output
    projection; the 1/rowsum scale and the folded output bias
    bo' = b_v @ W_out + b_out (valid because softmax rows sum to 1)
    are applied in ONE fused DVE scalar_tensor_tensor on the final
    PSUM->SBUF copy: out = (po * rd) + bo'.
  - q is only projected for the 2048 owned rows (not the halo).
  - attendedT accumulates the 4 e-chunks into a single PSUM bank using
    per-column-range start=True groups (start clears only the has_written
    bits bank-wide; prior ranges' values are complete and unaffected).
  - PE warm-up: a chain of small matmuls on a zeroed SBUF tile issued
    before any DMA-dependent work, so the HAM clock gate reaches 8/8
    before the real matmuls start, and a dummy Exp to preload the ACT
    table set during the DMA head.
"""

import sys

sys.path.insert(0, "/opt/trn_rl_repo")

import ml_dtypes
import numpy as np

import concourse.bass as bass  # noqa: F401  (registers types)
import concourse.tile as tile
from concourse import bacc, mybir
from concourse.bass_utils import run_bass_kernel_spmd

F32 = mybir.dt.float32
BF16 = mybir.dt.bfloat16
NPBF = ml_dtypes.bfloat16

B, S, E = 4, 4096, 512
WINDOW = 64
HALF = S // 2              # 2048 query rows per core
ROWS = HALF + 2 * WINDOW   # 2176 local rows incl. halo
EC = E // 128              # 4 contraction chunks
NT = HALF // 128           # 16 query subtiles per core
NCH = NT + 1               # 17 key chunks per core
WARMUP_MMS = 100

# exp_all column layout: chunk 0 -> 128 cols, chunks 1..15 -> 256, chunk 16 -> 128
OFF = [0] + [128 + 256 * (j - 1) for j in range(1, NCH)]
WID = [128] + [256] * 15 + [128]
EXPW = OFF[16] + 128       # 4096

# xT DRAM/SBUF layout: slice-major [s][e][w_s]; slices of the 2176 local rows
XSLICE = [(0, 512), (512, 512), (1024, 512), (1536, 512), (2048, 128)]
SOFF = [0, 512, 1024, 1536, 2048, 2176]   # per-e logical col offsets


def xcol(e, c0):
    """Column of local row c0 for e-chunk e in the slice-major xTa tile."""
    s = min(c0 // 512, 4)
    base, w = XSLICE[s]
    return 4 * base + e * w + (c0 - base)


_NC_CACHE = {}


def _qspan(j):
    """(first qT column, width) of key chunk j's query span."""
    if j == 0:
        return 0, 128
    if j == NCH - 1:
        return HALF - 128, 128
    return 128 * j - 128, 256


def _build():
    nc = bacc.Bacc("TRN2", target_bir_lowering=False, debug=False, num_devices=8)

    xT_d = nc.dram_tensor("xT", [128, 4 * ROWS], BF16, kind="ExternalInput")
    wqkv_d = nc.dram_tensor("wqkv", [128, 12 * E], BF16, kind="ExternalInput")
    bq_d = nc.dram_tensor("bq", [128, 8], F32, kind="ExternalInput")
    id_d = nc.dram_tensor("ident", [128, 128], BF16, kind="ExternalInput")
    bob_d = nc.dram_tensor("bob", [128, E], F32, kind="ExternalInput")
    ones_d = nc.dram_tensor("ones", [128, 1], BF16, kind="ExternalInput")
    mask_d = nc.dram_tensor("masks", [128, 512], BF16, kind="ExternalInput")
    out_d = nc.dram_tensor("out", [HALF, E], BF16, kind="ExternalOutput")

    ACT = mybir.ActivationFunctionType
    ALU = mybir.AluOpType

    with tile.TileContext(nc) as tc:
        with (
            tc.tile_pool(name="const", bufs=1) as const,
            tc.tile_pool(name="big", bufs=1) as big,
            tc.tile_pool(name="attn", bufs=2) as attn,
            tc.tile_pool(name="ps", bufs=1, space="PSUM") as psp,
        ):
            # ---- SBUF tiles ----
            wqa = const.tile([128, 12 * E], BF16, name="wqa", tag="wqa")
            id_sb = const.tile([128, 128], BF16, name="idn", tag="idn")
            bq_sb = const.tile([128, 8], F32, name="bq", tag="bq")
            bob_sb = const.tile([128, E], F32, name="bob", tag="bob")
            ones_sb = const.tile([128, 1], BF16, name="ones1", tag="ones1")
            mask_sb = const.tile([128, 512], BF16, name="msk", tag="msk")
            warm_sb = const.tile([128, 64], BF16, name="warm", tag="warm")
            dummy_sb = const.tile([128, 1], F32, name="dmy", tag="dmy")
            rd_all = const.tile([128, NT], F32, name="rd", tag="rd")

            xTa = big.tile([128, 4 * ROWS], BF16, name="xTa", tag="xTa")
            qT = [big.tile([128, HALF], BF16, name=f"qT{f}", tag=f"qT{f}")
                  for f in range(EC)]
            kT = [big.tile([128, ROWS], BF16, name=f"kT{f}", tag=f"kT{f}")
                  for f in range(EC)]
            v_sb = [big.tile([128, E], BF16, name=f"v{r}", tag=f"v{r}")
                    for r in range(NCH)]
            exp_all = big.tile([128, EXPW], BF16, name="expa", tag="expa")

            # ---- PE warm-up + ACT table preload (no DMA deps) ----
            nc.vector.memset(warm_sb[:], 0.0)
            nc.scalar.activation(out=dummy_sb[:], in_=warm_sb[:, 0:1],
                                 func=ACT.Exp)
            pw = psp.tile([128, 64], F32, name="pwarm", tag="o", bufs=2)
            for i in range(WARMUP_MMS):
                nc.tensor.matmul(pw[0:64, :], warm_sb[:, 0:64],
                                 warm_sb[:, 0:64], start=True, stop=True)

            # ---- input DMA ----
            # One DMA engine round-robins the queues at transfer granularity,
            # so the critical-path stream (q weights, then xT slices, then
            # k/v weights) goes in order on ONE queue; only tiny constants
            # ride the scalar queue.  Output DMAs use the gpsimd queue.
            nc.scalar.dma_start(out=bq_sb, in_=bq_d[:, :])
            nc.scalar.dma_start(out=mask_sb, in_=mask_d[:, :])
            nc.scalar.dma_start(out=ones_sb, in_=ones_d[:, :])
            nc.scalar.dma_start(out=id_sb, in_=id_d[:, :])
            nc.sync.dma_start(out=wqa[:, 0:512], in_=wqkv_d[:, 0:512])
            nc.sync.dma_start(out=xTa[:, 0:SOFF[1] * 4],
                              in_=xT_d[:, 0:SOFF[1] * 4])
            for f in range(1, EC):
                nc.sync.dma_start(out=wqa[:, 512 * f:512 * (f + 1)],
                                  in_=wqkv_d[:, 512 * f:512 * (f + 1)])
            for s in range(1, 5):
                a, b = SOFF[s] * 4, SOFF[s + 1] * 4
                nc.sync.dma_start(out=xTa[:, a:b], in_=xT_d[:, a:b])
            nc.sync.dma_start(out=wqa[:, 4 * 512:8 * 512],
                              in_=wqkv_d[:, 4 * 512:8 * 512])
            nc.sync.dma_start(out=wqa[:, 8 * 512:12 * 512],
                              in_=wqkv_d[:, 8 * 512:12 * 512])
            nc.sync.dma_start(out=bob_sb, in_=bob_d[:, :])

            # ---- q projection: slice-aligned groups ----
            # qT[f][:, c] = q[row 64 + c, 128 f + p]
            QG = [(64, 448), (512, 512), (1024, 512), (1536, 512), (2048, 64)]
            for (r0, w) in QG:
                for f in range(EC):
                    ps = psp.tile([128, 512], F32, name=f"pq{r0}_{f}",
                                  tag="pp", bufs=2)
                    for e in range(EC):
                        nc.tensor.matmul(
                            ps[:, :w],
                            wqa[:, 512 * f + 128 * e:512 * f + 128 * (e + 1)],
                            xTa[:, xcol(e, r0):xcol(e, r0) + w],
                            start=(e == 0), stop=(e == EC - 1),
                        )
                    nc.scalar.activation(
                        out=qT[f][:, r0 - 64:r0 - 64 + w], in_=ps[:, :w],
                        func=ACT.Identity, bias=bq_sb[:, f:f + 1],
                    )

            # ---- k projection over all 2176 rows ----
            KS = [(0, 512), (512, 512), (1024, 512), (1536, 512), (2048, 128)]
            for (c0, w) in KS:
                for f in range(EC):
                    ps = psp.tile([128, 512], F32, name=f"pk{c0}_{f}",
                                  tag="pp", bufs=2)
                    for e in range(EC):
                        nc.tensor.matmul(
                            ps[:, :w],
                            wqa[:, 2048 + 512 * e + 128 * f:
                                 2048 + 512 * e + 128 * (f + 1)],
                            xTa[:, xcol(e, c0):xcol(e, c0) + w],
                            start=(e == 0), stop=(e == EC - 1),
                        )
                    nc.vector.tensor_scalar_add(
                        kT[f][:, c0:c0 + w], ps[:, :w],
                        bq_sb[:, 4 + f:5 + f],
                    )

            # ---- v projection (natural layout, NO bias — folded into bo') ----
            for r in range(NCH):
                ps = psp.tile([128, 512], F32, name=f"pv{r}", tag="pp", bufs=2)
                for e in range(EC):
                    nc.tensor.matmul(
                        ps[:],
                        xTa[:, xcol(e, 128 * r):xcol(e, 128 * r) + 128],
                        wqa[:, 4096 + 512 * e:4096 + 512 * (e + 1)],
                        start=(e == 0), stop=(e == EC - 1),
                    )
                nc.vector.tensor_copy(v_sb[r][:], ps[:])

            # ---- attention ----
            def emit_scores(j):
                c0, w = _qspan(j)
                ps = psp.tile([128, 256], F32, name=f"ps_s{j}", tag="sra",
                              bufs=3)
                moff = 0 if j == 0 else (384 if j == NCH - 1 else 128)
                nc.tensor.matmul(ps[:, :w], id_sb[:],
                                 mask_sb[:, moff:moff + w],
                                 start=True, stop=False)
                for e in range(EC):
                    nc.tensor.matmul(
                        ps[:, :w],
                        kT[e][:, 128 * j:128 * (j + 1)],
                        qT[e][:, c0:c0 + w],
                        start=False, stop=(e == EC - 1),
                    )
                nc.scalar.activation(out=exp_all[:, OFF[j]:OFF[j] + w],
                                     in_=ps[:, :w], func=ACT.Exp)

            def emit_subtile(t):
                lo = OFF[t] if t == 0 else OFF[t] + 128
                ro = OFF[t + 1]
                # row sums [q, 1] with exp tiles stationary
                pr = psp.tile([128, 1], F32, name=f"ps_r{t}", tag="r",
                              bufs=1)
                nc.tensor.matmul(pr[:], exp_all[:, lo:lo + 128], ones_sb[:],
                                 start=True, stop=False)
                nc.tensor.matmul(pr[:], exp_all[:, ro:ro + 128], ones_sb[:],
                                 start=False, stop=True)
                nc.vector.reciprocal(rd_all[:, t:t + 1], pr[:])
                # merged attended + output projection:
                # out[q, :] = sum_k exp[k, q] * vwo[k, :]   (vwo = v @ W_out)
                po = psp.tile([128, 512], F32, name=f"ps_o{t}", tag="o",
                              bufs=2)
                nc.tensor.matmul(po[:], exp_all[:, lo:lo + 128], v_sb[t][:],
                                 start=True, stop=False)
                nc.tensor.matmul(po[:], exp_all[:, ro:ro + 128],
                                 v_sb[t + 1][:], start=False, stop=True)
                ost = attn.tile([128, 512], BF16, name=f"ost{t}", tag="ost")
                nc.vector.scalar_tensor_tensor(
                    ost[:], po[:], rd_all[:, t:t + 1], bob_sb[:],
                    ALU.mult, ALU.add,
                )
                nc.gpsimd.dma_start(out=out_d[128 * t:128 * (t + 1), :],
                                    in_=ost[:])

            # subtiles 0-2 are held back as tail filler: by the time the
            # last scores finish, their dependencies are long resolved, so
            # they hide the final subtiles' exp/recip latencies.
            for j in range(NCH):
                emit_scores(j)
                if j >= 6:
                    emit_subtile(j - 3)
            for t in (14, 0, 15, 1, 2):
                emit_subtile(t)

    nc.compile()
    return nc


def _get_nc():
    if "nc" not in _NC_CACHE:
        _NC_CACHE["nc"] = _build()
    return _NC_CACHE["nc"]


def _prep_shared(W_qkv, b_qkv, W_out, b_out):
    scale = 1.0 / np.sqrt(np.float32(E))
    w = np.array(W_qkv, dtype=np.float32, copy=True)
    w[:, :E] *= scale
    b = np.array(b_qkv, dtype=np.float32, copy=True)
    b[:E] *= scale
    bq_col = np.empty((128, 8), dtype=np.float32)
    for f in range(EC):
        bq_col[:, f] = b[128 * f:128 * (f + 1)]
        bq_col[:, 4 + f] = b[E + 128 * f:E + 128 * (f + 1)]
    b_v = b[2 * E:]
    bo_p = (b_v @ np.asarray(W_out, np.float32)
            + np.asarray(b_out, np.float32)).astype(np.float32)
    # fold W_out into the v projection: vwo = x @ (W_v @ W_out)
    wvo = (np.asarray(w[:, 2 * E:], np.float64)
           @ np.asarray(W_out, np.float64)).astype(np.float32)
    # pack [512 x (q|k|vwo)] -> [128, 12*512]:
    #   q block f-major [f][e][128], then k and vwo blocks e-major [e][512]
    wb = [w[128 * e:128 * (e + 1), 128 * f:128 * (f + 1)]
          for f in range(EC) for e in range(EC)]
    wb += [w[128 * e:128 * (e + 1), E:2 * E] for e in range(EC)]
    wb += [wvo[128 * e:128 * (e + 1), :] for e in range(EC)]
    wqkv_p = np.concatenate(wb, axis=1)
    return {
        "wqkv": np.ascontiguousarray(wqkv_p.astype(NPBF)),
        "bq": np.ascontiguousarray(bq_col),
        "ident": np.ascontiguousarray(np.eye(128, dtype=NPBF)),
        "bob": np.ascontiguousarray(np.tile(bo_p[None, :], (128, 1))),
        "ones": np.ones((128, 1), dtype=NPBF),
    }


def _masks_for(h: int) -> np.ndarray:
    """Additive masks [128, 512] bf16: [chunk0 | interior | chunk16].

    Tile element (a, c) of chunk j is key local-row L = 128j + a against
    query local-row r = span_start(j) + 64 + c... computed from first
    principles below.  Valid iff |global q - global k| <= WINDOW and the
    key's global position is inside [0, S)."""
    NEG = np.float32(-1e30)

    def chunk_mask(j, h):
        c0, w = _qspan(j)
        L = 128 * j + np.arange(128)[:, None]      # local key row
        r = 64 + (c0 + np.arange(w))[None, :]      # local query row
        valid = np.abs(r - L) <= WINDOW
        if h == 0:
            valid = valid & (L >= WINDOW)          # global key >= 0
        else:
            valid = valid & (L < ROWS - WINDOW)    # global key < S
        return np.where(valid, np.float32(0.0), NEG)

    m0 = chunk_mask(0, h)                  # [128, 128]
    mi = chunk_mask(8, h)                  # interior pattern, j-independent
    m16 = chunk_mask(NCH - 1, h)           # [128, 128]
    return np.ascontiguousarray(
        np.concatenate([m0, mi, m16], axis=1).astype(NPBF))


def _install_ntff_shim():
    """The agent image's antenv lacks axon_hooks; synthesize it from the
    boot module's ctypes NTFF driver so trace=True can capture HW timing."""
    import types
    if "antenv.axon_hooks" in sys.modules:
        return
    try:
        from trn_agent_boot.trn_boot import _ntff_profile_via_ctypes
        hook = _ntff_profile_via_ctypes("/opt/axon/libaxon_pjrt.so")
    except Exception:
        hook = None
    mod = types.ModuleType("antenv.axon_hooks")
    mod.get_axon_ntff_profile_hook = lambda: hook
    mod.set_axon_ntff_profile_hook = lambda h: None
    sys.modules["antenv.axon_hooks"] = mod
    # avoid S3 artifact upload attempts during local profile processing
    try:
        from concourse import bass_utils as _bu
        _bu.upload_artifacts = lambda tmpdir: tmpdir
    except Exception:
        pass


def kernel(x, W_qkv, b_qkv, W_out, b_out, _trace=False):
    x = np.asarray(x, dtype=np.float32)
    nc = _get_nc()
    shared = _prep_shared(W_qkv, b_qkv, W_out, b_out)
    masks = [_masks_for(0), _masks_for(1)]

    in_maps = []
    for core in range(8):
        b, h = divmod(core, 2)
        lo = h * HALF - WINDOW
        hi = lo + ROWS
        xh = np.zeros((ROWS, E), dtype=np.float32)
        s0, s1 = max(lo, 0), min(hi, S)
        xh[s0 - lo:s1 - lo] = x[b, s0:s1]
        xt = xh.T  # [E, ROWS]
        # slice-major pack [s][e][w] -> [128, 4*ROWS]
        blocks = [xt[128 * e:128 * (e + 1), c0:c0 + w]
                  for (c0, w) in XSLICE for e in range(EC)]
        xt_p = np.concatenate(blocks, axis=1)
        in_maps.append({
            "xT": np.ascontiguousarray(xt_p.astype(NPBF)),
            "masks": masks[h],
            **shared,
        })

    kwargs = {}
    if _trace:
        _install_ntff_shim()
        kwargs = dict(trace=True, trace_cores=[0])
    res = run_bass_kernel_spmd(nc, in_maps, core_ids=list(range(8)), **kwargs)

    out = np.empty((B, S, E), dtype=np.float32)
    for core in range(8):
        b, h = divmod(core, 2)
        out[b, h * HALF:(h + 1) * HALF] = \
            res.results[core]["out"].astype(np.float32)
    if _trace:
        return out, res
    return out
